# revision 20
# baseline (speedup 1.0000x reference)
"""Multi-head attention (no qkv proj) + out_proj, sharded over 8 TRN2 cores.

Sharding: core i handles batch b = i//4, query rows tc = (i//2)%2 of 512,
and head group hg = i%2 (8 of 16 heads).  out_proj weight is column-sharded
over the head groups; the "all-reduce" is a host-side partial-sum of the two
head-group outputs at gather time.

v3 pipeline. Engine budget per core (all under the ~41us DMA floor):
  ACT  ~34us: exp(scores-2) only, in wide windows (3- or 2-chunk = 1536/1024
              cols per instruction) alternating between two PSUM score
              buffers (3+2 banks) so ACT never waits on banks.
  DVE  ~34us: expv = exp_out * exp(bias) (host-precomputed fp16, masked
              rows exactly 0), plus per-head softmax-normalize chain.
  PE   ~36us: QK scores, AV (augmented 65th row = denominator), K=1 rcp
              broadcast, out_proj. AV trails exp by 2 windows so the PE
              FIFO never stalls on ACT/DVE.
  DMA  ~41us: 13.9MB fp16 (bias 8.4MB dominates), issued densely.
PSUM: scoresA 3 banks + scoresB 2 banks + av/bc pool 3 banks = 8.
"""

import numpy as np

import concourse.mybir as mybir
import concourse.tile as tile
from concourse import bacc
from concourse.bass_utils import run_bass_kernel_spmd

F32 = mybir.dt.float32
F16 = mybir.dt.float16
NP16 = np.float16

P = 128          # partitions
T = 512          # query rows per core
S = 1024         # key length
H = 8            # heads per core (of 16)
HD = 64          # head dim
DIN = H * HD     # local d_model slice (512)
NDIN = DIN // P  # 4 chunks
DM = 1024        # full d_model
NS = S // P      # 8 s-chunks per head
ND = DM // P     # 8 d_out chunks
NCHUNK = H * NS  # 64 (head, s-chunk) pairs
SCALE = HD ** -0.5
EXP_SHIFT = -2.0

AF = mybir.ActivationFunctionType

# exp windows: alternate 3-chunk (1536 col) / 2-chunk (1024 col) instructions
WINS = []
_g, _i = 0, 0
while _g < NCHUNK:
    n = min(3 if _i % 2 == 0 else 2, NCHUNK - _g)
    WINS.append((_g, n))
    _g += n
    _i += 1
NWIN = len(WINS)
AV_LAG = 1


def build_bass():
    nc = bacc.Bacc()

    qT_d = nc.dram_tensor("qT", [DIN, T], F16, kind="ExternalInput")
    kT_d = nc.dram_tensor("kT", [DIN, S], F16, kind="ExternalInput")
    vaug_d = nc.dram_tensor("vaug", [S, H * (HD + 1)], F16, kind="ExternalInput")
    ebT_d = nc.dram_tensor("ebT", [H, S, T], F16, kind="ExternalInput")
    wT_d = nc.dram_tensor("wT", [DIN, DM], F16, kind="ExternalInput")
    outT_d = nc.dram_tensor("outT", [DM, T], F16, kind="ExternalOutput")

    with tile.TileContext(nc) as tc, nc.allow_low_precision(reason="fp16 pipeline"):
        with (
            tc.tile_pool(name="weights", bufs=1) as wpool,
            tc.tile_pool(name="bias", bufs=5) as bpool,
            tc.tile_pool(name="eo", bufs=3) as eopool,
            tc.tile_pool(name="ev", bufs=4) as evpool,
            tc.tile_pool(name="small", bufs=3) as spool,
            tc.tile_pool(name="osb", bufs=3) as opool_sb,
        ):
            qT_t = [wpool.tile([P, T], F16, name=f"qT{c}", tag=f"qT{c}") for c in range(NDIN)]
            kT_t = [wpool.tile([P, S], F16, name=f"kT{c}", tag=f"kT{c}") for c in range(NDIN)]
            vaug_t = wpool.tile([P, NS * H * (HD + 1)], F16, name="vaug", tag="vaug")
            wT_t = [wpool.tile([P, DM], F16, name=f"wT{c}", tag=f"wT{c}") for c in range(NDIN)]
            ones_t = wpool.tile([1, HD], F16, name="ones", tag="ones")
            nc.vector.memset(ones_t[:], 1.0)
            eshift_t = wpool.tile([P, 1], F32, name="eshift", tag="eshift")
            nc.vector.memset(eshift_t[:], EXP_SHIFT)
            warm_t = wpool.tile([P, T], F16, name="warm", tag="warm")
            nc.vector.memset(warm_t[:], 0.0)
            aflat_t = [wpool.tile([P, T], F16, name=f"af{c}", tag=f"af{c}") for c in range(NDIN)]

            eb_t = {}

            def eb_dma(h, qt):
                # one quarter = two s-chunk DMAs (sem granularity: QK waits
                # only its own 128KB slice)
                if h not in eb_t:
                    eb_t[h] = bpool.tile([P, NS * T], F16, name=f"eb{h}", tag="eb")
                for e in (2 * qt, 2 * qt + 1):
                    nc.sync.dma_start(
                        out=eb_t[h][:].rearrange("p (e t) -> p e t", e=8)[:, e],
                        in_=ebT_d[h, :, :].rearrange("(e p) t -> p e t", p=P)[:, e],
                    )

            def vaug_dma(qt):
                nc.sync.dma_start(
                    out=vaug_t[:].rearrange("p (qt sc x) -> p qt sc x", qt=4, sc=NS // 4)[:, qt],
                    in_=vaug_d[:, :].rearrange("(qt sc p) x -> p qt sc x", p=P, qt=4)[:, qt],
                )

            # ---- pre-loop DMAs (issue order == queue order) ----
            nc.sync.dma_start(out=qT_t[0][:], in_=qT_d[0:P, :])
            nc.sync.dma_start(out=kT_t[0][:], in_=kT_d[0:P, :])
            eb_dma(0, 0)
            eb_dma(0, 1)
            vaug_dma(0)
            eb_dma(0, 2)
            vaug_dma(1)
            eb_dma(0, 3)
            vaug_dma(2)
            eb_dma(1, 0)
            vaug_dma(3)
            for qt in range(1, 4):
                eb_dma(1, qt)
            nc.sync.dma_start(out=kT_t[1][:], in_=kT_d[P:2 * P, :])
            nc.sync.dma_start(out=qT_t[1][:], in_=qT_d[P:2 * P, :])
            for qt in range(4):
                eb_dma(2, qt)

            touched = set([0, 1, 2])

            def on_head_start(h):
                hp2 = h + 3
                if hp2 < H and hp2 not in touched:
                    touched.add(hp2)
                    for qt in range(4):
                        eb_dma(hp2, qt)
                if h % 2 == 1 and h // 2 + 2 < NDIN:
                    c = h // 2 + 2
                    nc.sync.dma_start(out=kT_t[c][:], in_=kT_d[c * P:(c + 1) * P, :])
                    nc.sync.dma_start(out=qT_t[c][:], in_=qT_d[c * P:(c + 1) * P, :])
                if 3 <= h <= 6:
                    c = h - 3
                    nc.sync.dma_start(out=wT_t[c][:], in_=wT_d[c * P:(c + 1) * P, :])

            with (
                tc.tile_pool(name="scA", bufs=1, space="PSUM") as scpsA,
                tc.tile_pool(name="scB", bufs=1, space="PSUM") as scpsB,
                tc.tile_pool(name="avps", bufs=3, space="PSUM") as avps,
            ):
                # HAM warm-up: dummy matmuls un-throttle the PE clock
                # (4/8 -> 8/8) while the first DMAs land.
                wm_ps = scpsB.tile([P, 2 * T], F32, name="wm", tag="scB")
                for _ in range(10):
                    nc.tensor.matmul(wm_ps[:, 0:T], warm_t[:, 0:P], warm_t[:],
                                     start=True, stop=True)

                expv_w = [None] * NWIN
                av_t = {}
                pending = []   # heads awaiting bcast+normalize
                chain = {}     # h -> rcp tile

                def emit_qk(w):
                    g0, n = WINS[w]
                    width = 1536 if w % 2 == 0 else 1024
                    pool = scpsA if w % 2 == 0 else scpsB
                    sc_t = pool.tile([P, width], F32, name=f"sc{w}",
                                     tag=f"sc{'A' if w % 2 == 0 else 'B'}")
                    for j in range(n):
                        g = g0 + j
                        h, sc = g // 8, g % 8
                        if sc == 0:
                            on_head_start(h)
                        c2, half = divmod(h, 2)
                        hp = slice(half * HD, (half + 1) * HD)
                        nc.tensor.matmul(
                            sc_t[:, j * T:(j + 1) * T],
                            kT_t[c2][hp, sc * P:(sc + 1) * P],
                            qT_t[c2][hp, :],
                            start=True, stop=True,
                        )
                    eo = eopool.tile([P, 1536], F16, name=f"eo{w}", tag="eo")
                    nc.scalar.activation(
                        eo[:, 0:n * T], sc_t[:, 0:n * T], AF.Exp,
                        bias=eshift_t[:], scale=1.0,
                    )
                    # expv = exp(scores) * exp(bias): one DVE op per
                    # contiguous same-head run of chunks
                    ev = evpool.tile([P, 1536], F16, name=f"ev{w}", tag="ev")
                    expv_w[w] = ev
                    j = 0
                    while j < n:
                        h = (g0 + j) // 8
                        j2 = j
                        while j2 < n and (g0 + j2) // 8 == h:
                            j2 += 1
                        sl = slice(j * T, j2 * T)
                        bsl = slice((g0 + j) % NS * T, ((g0 + j) % NS + j2 - j) * T)
                        nc.vector.tensor_mul(ev[:, sl], eo[:, sl], eb_t[h][:, bsl])
                        j = j2

                def emit_bcast_flush(bc_pool=None):
                    while pending:
                        h = pending.pop(0)
                        rcp = chain.pop(h)
                        av = av_t[h]
                        rcp16 = spool.tile([1, T], F16, name=f"rh{h}", tag="rcp16")
                        nc.vector.tensor_copy(rcp16[:], rcp[:])
                        if bc_pool is None:
                            bc = avps.tile([P, T], F32, name=f"bc{h}", tag="avbc")
                        else:
                            bc = bc_pool.tile([P, 2 * T], F32, name=f"bc{h}", tag="scB")
                        nc.tensor.matmul(
                            bc[0:HD, :], ones_t[0:1, :], rcp16[:],
                            start=True, stop=True,
                        )
                        bc_sb = spool.tile([HD, T], F32, name=f"bcs{h}", tag="bc_sb")
                        if h >= 6:
                            nc.scalar.copy(bc_sb[:], bc[0:HD, :])
                        else:
                            nc.vector.tensor_copy(bc_sb[:], bc[0:HD, :])
                        c2, half = divmod(h, 2)
                        hp = slice(half * HD, (half + 1) * HD)
                        nc.vector.tensor_mul(aflat_t[c2][hp, :], av[0:HD, :], bc_sb[:])

                def emit_av(w):
                    g0, n = WINS[w]
                    for j in range(n):
                        g = g0 + j
                        h, sc = g // 8, g % 8
                        if sc == 0:
                            av_t[h] = avps.tile([P, T], F32, name=f"av{h}", tag="avbc")
                        nc.tensor.matmul(
                            av_t[h][0:HD + 1, :],
                            vaug_t[:, (sc * H + h) * (HD + 1):(sc * H + h + 1) * (HD + 1)],
                            expv_w[w][:, j * T:(j + 1) * T],
                            start=(sc == 0), stop=(sc == NS - 1),
                            skip_group_check=True,
                        )
                        if sc == NS - 1:
                            den_sb = spool.tile([1, T], F32, name=f"dn{h}", tag="den")
                            if h >= 6:
                                nc.scalar.copy(den_sb[:], av_t[h][HD:HD + 1, :])
                            else:
                                nc.vector.tensor_copy(den_sb[:], av_t[h][HD:HD + 1, :])
                            rcp = spool.tile([1, T], F32, name=f"rc{h}", tag="rcp")
                            nc.vector.reciprocal_approx_fast(rcp[:], den_sb[:])
                            chain[h] = rcp
                            pending.append(h)

                def keep_alive(n):
                    # tiny FD=128 dummies keep the HAM clock gate at 8/8
                    for _ in range(n):
                        nc.tensor.matmul(wm_ps[:, 0:P], warm_t[:, 0:P],
                                         warm_t[:, 0:P], start=True, stop=True)

                for w in range(NWIN):
                    if w % 2 == 1:
                        keep_alive(4)
                    emit_qk(w)
                    if w >= AV_LAG:
                        emit_av(w - AV_LAG)
                    emit_bcast_flush()

                # ---- out_proj drain, interleaved with the last head's
                # AV + normalize. Accumulators live in the freed score banks:
                # scA tile = dc0-2, scB tile = dc3-4, avps tiles = dc5-7.
                oA = scpsA.tile([P, 1536], F32, name="oA", tag="scA")
                oB = scpsB.tile([P, 2 * T], F32, name="oB", tag="scB")
                o_sl = {}
                for dc in range(3):
                    o_sl[dc] = oA[:, dc * T:(dc + 1) * T]
                for dc in (3, 4):
                    o_sl[dc] = oB[:, (dc - 3) * T:(dc - 2) * T]

                def emit_partials(dc):
                    for dinc in range(NDIN - 1):
                        nc.tensor.matmul(
                            o_sl[dc],
                            wT_t[dinc][:, dc * P:(dc + 1) * P],
                            aflat_t[dinc][:],
                            start=(dinc == 0), stop=False,
                            skip_group_check=True,
                        )

                emit_partials(0)                   # fills PE during exp(25)
                emit_av(NWIN - 1)                  # last AV chunk + den/rcp
                emit_partials(1)
                emit_partials(2)
                emit_partials(3)
                emit_partials(4)
                emit_bcast_flush()                 # h7: cast/bcast/evac/aflat
                for dc in (5, 6, 7):
                    o_sl[dc] = avps.tile([P, T], F32, name=f"o{dc}", tag="avbc")[:]
                    emit_partials(dc)
                for dc in range(ND):
                    nc.tensor.matmul(
                        o_sl[dc],
                        wT_t[NDIN - 1][:, dc * P:(dc + 1) * P],
                        aflat_t[NDIN - 1][:],
                        start=False, stop=True,
                        skip_group_check=True,
                    )
                    osb = opool_sb.tile([P, T], F16, name=f"osb{dc}", tag="osb")
                    if dc % 2 == 0:
                        nc.scalar.copy(osb[:], o_sl[dc])
                    else:
                        nc.vector.tensor_copy(osb[:], o_sl[dc])
                    nc.sync.dma_start(
                        out=outT_d[dc * P:(dc + 1) * P, :],
                        in_=osb[:],
                    )

    nc.finalize()
    return nc



_NC = None


def _get_nc():
    global _NC
    if _NC is None:
        _NC = build_bass()
    return _NC


def _core_index(b, tc_i, hg):
    return b * 4 + tc_i * 2 + hg


def _make_in_maps(query, key, value, attn_bias, key_padding_mask, out_w, out_b):
    query = np.asarray(query, dtype=np.float32)
    key = np.asarray(key, dtype=np.float32)
    value = np.asarray(value, dtype=np.float32)
    attn_bias = np.asarray(attn_bias, dtype=np.float32)
    mask = np.asarray(key_padding_mask).astype(bool)
    out_w = np.asarray(out_w, dtype=np.float32)

    wT_full = np.ascontiguousarray(out_w.T).astype(NP16)   # [din, dout]

    maps = [None] * 8
    for b in range(2):
        kT_full = np.ascontiguousarray(key[b].T).astype(NP16)  # [1024, 1024]
        for hg in range(2):
            hs = hg * H              # first global head of the group
            ds = hg * DIN            # first d_model row of the group
            vaug = np.ones((S, H * (HD + 1)), NP16)
            vaug.reshape(S, H, HD + 1)[:, :, :HD] = (
                value[b, :, ds:ds + DIN].reshape(S, H, HD))
            kT = np.ascontiguousarray(kT_full[ds:ds + DIN])
            wT = np.ascontiguousarray(wT_full[ds:ds + DIN])
            for tc_i in range(2):
                t0 = tc_i * T
                qT = np.ascontiguousarray(
                    (query[b, t0:t0 + T, ds:ds + DIN].T * SCALE)).astype(NP16)
                ebT = np.ascontiguousarray(
                    np.clip(attn_bias[b, hs:hs + H, t0:t0 + T, :], -10.0, 10.0)
                    .transpose(0, 2, 1))
                ebT[:, mask[b], :] = -np.inf
                np.exp(ebT, out=ebT)      # masked rows -> exactly 0
                maps[_core_index(b, tc_i, hg)] = {
                    "qT": qT, "kT": kT, "vaug": vaug,
                    "ebT": ebT.astype(NP16),
                    "wT": wT,
                }
    return maps


def run(inputs, trace=False, **run_kwargs):
    """Returns (output [2,1024,1024] f32, BassKernelResults)."""
    nc = _get_nc()
    inputs_out_b = inputs["out_b"]
    in_maps = _make_in_maps(**inputs)
    res = run_bass_kernel_spmd(
        nc, in_maps, core_ids=list(range(8)), trace=trace, **run_kwargs
    )
    out = np.empty((2, S, DM), np.float32)
    for b in range(2):
        for tc_i in range(2):
            part = (np.asarray(res.results[_core_index(b, tc_i, 0)]["outT"], dtype=np.float32)
                    + np.asarray(res.results[_core_index(b, tc_i, 1)]["outT"], dtype=np.float32))
            out[b, tc_i * T:(tc_i + 1) * T, :] = part.T + np.asarray(inputs_out_b, dtype=np.float32)[None, :]
    return out, res


def kernel(**inputs):
    out, _ = run(inputs, trace=False)
    return out


# revision 21
# speedup vs baseline: 1.0928x; 1.0928x over previous
"""Multi-head attention (no qkv proj) + out_proj, sharded over 8 TRN2 cores.

Sharding: core i handles batch b = i//4, query rows tc = (i//2)%2 of 512,
and head group hg = i%2 (8 of 16 heads).  out_proj weight is column-sharded
over the head groups; the "all-reduce" is a host-side partial-sum of the two
head-group outputs at gather time.

v3 pipeline. Engine budget per core (all under the ~41us DMA floor):
  ACT  ~34us: exp(scores-2) only, in wide windows (3- or 2-chunk = 1536/1024
              cols per instruction) alternating between two PSUM score
              buffers (3+2 banks) so ACT never waits on banks.
  DVE  ~34us: expv = exp_out * exp(bias) (host-precomputed fp16, masked
              rows exactly 0), plus per-head softmax-normalize chain.
  PE   ~36us: QK scores, AV (augmented 65th row = denominator), K=1 rcp
              broadcast, out_proj. AV trails exp by 2 windows so the PE
              FIFO never stalls on ACT/DVE.
  DMA  ~41us: 13.9MB fp16 (bias 8.4MB dominates), issued densely.
PSUM: scoresA 3 banks + scoresB 2 banks + av/bc pool 3 banks = 8.
"""

import numpy as np

import concourse.mybir as mybir
import concourse.tile as tile
from concourse import bacc
from concourse.bass_utils import run_bass_kernel_spmd

F32 = mybir.dt.float32
F16 = mybir.dt.float16
NP16 = np.float16

P = 128          # partitions
T = 512          # query rows per core
S = 1024         # key length
H = 8            # heads per core (of 16)
HD = 64          # head dim
DIN = H * HD     # local d_model slice (512)
NDIN = DIN // P  # 4 chunks
DM = 1024        # full d_model
NS = S // P      # 8 s-chunks per head
ND = DM // P     # 8 d_out chunks
NCHUNK = H * NS  # 64 (head, s-chunk) pairs
SCALE = HD ** -0.5
EXP_SHIFT = -2.0

AF = mybir.ActivationFunctionType

# exp windows: alternate 3-chunk (1536 col) / 2-chunk (1024 col) instructions
WINS = []
_g, _i = 0, 0
while _g < NCHUNK:
    n = min(3 if _i % 2 == 0 else 2, NCHUNK - _g)
    WINS.append((_g, n))
    _g += n
    _i += 1
NWIN = len(WINS)
AV_LAG = 1


def build_bass():
    nc = bacc.Bacc()

    qT_d = nc.dram_tensor("qT", [DIN, T], F16, kind="ExternalInput")
    kT_d = nc.dram_tensor("kT", [DIN, S], F16, kind="ExternalInput")
    vaug_d = nc.dram_tensor("vaug", [S, H * (HD + 1)], F16, kind="ExternalInput")
    ebT_d = nc.dram_tensor("ebT", [H, S, T], F16, kind="ExternalInput")
    wT_d = nc.dram_tensor("wT", [DIN, DM], F16, kind="ExternalInput")
    outT_d = nc.dram_tensor("outT", [DM, T], F16, kind="ExternalOutput")

    with tile.TileContext(nc) as tc, nc.allow_low_precision(reason="fp16 pipeline"):
        with (
            tc.tile_pool(name="weights", bufs=1) as wpool,
            tc.tile_pool(name="bias", bufs=5) as bpool,
            tc.tile_pool(name="eo", bufs=3) as eopool,
            tc.tile_pool(name="ev", bufs=4) as evpool,
            tc.tile_pool(name="small", bufs=3) as spool,
            tc.tile_pool(name="osb", bufs=3) as opool_sb,
        ):
            qT_t = [wpool.tile([P, T], F16, name=f"qT{c}", tag=f"qT{c}") for c in range(NDIN)]
            kT_t = [wpool.tile([P, S], F16, name=f"kT{c}", tag=f"kT{c}") for c in range(NDIN)]
            vaug_t = wpool.tile([P, NS * H * (HD + 1)], F16, name="vaug", tag="vaug")
            wT_t = [wpool.tile([P, DM], F16, name=f"wT{c}", tag=f"wT{c}") for c in range(NDIN)]
            ones_t = wpool.tile([1, HD], F16, name="ones", tag="ones")
            nc.vector.memset(ones_t[:], 1.0)
            eshift_t = wpool.tile([P, 1], F32, name="eshift", tag="eshift")
            nc.vector.memset(eshift_t[:], EXP_SHIFT)
            warm_t = wpool.tile([P, T], F16, name="warm", tag="warm")
            nc.vector.memset(warm_t[:], 0.0)
            aflat_t = [wpool.tile([P, T], F16, name=f"af{c}", tag=f"af{c}") for c in range(NDIN)]

            eb_t = {}

            def eb_dma(h, qt):
                if h not in eb_t:
                    eb_t[h] = bpool.tile([P, NS * T], F16, name=f"eb{h}", tag="eb")
                nc.sync.dma_start(
                    out=eb_t[h][:].rearrange("p (qt sc t) -> p qt sc t", qt=4, t=T)[:, qt],
                    in_=ebT_d[h, :, :].rearrange("(qt sc p) t -> p qt sc t", p=P, qt=4)[:, qt],
                )

            def vaug_dma(qt):
                nc.sync.dma_start(
                    out=vaug_t[:].rearrange("p (qt sc x) -> p qt sc x", qt=4, sc=NS // 4)[:, qt],
                    in_=vaug_d[:, :].rearrange("(qt sc p) x -> p qt sc x", p=P, qt=4)[:, qt],
                )

            # ---- pre-loop DMAs (issue order == queue order) ----
            nc.sync.dma_start(out=qT_t[0][:], in_=qT_d[0:P, :])
            nc.sync.dma_start(out=kT_t[0][:], in_=kT_d[0:P, :])
            eb_dma(0, 0)
            eb_dma(0, 1)
            vaug_dma(0)
            eb_dma(0, 2)
            vaug_dma(1)
            eb_dma(0, 3)
            vaug_dma(2)
            eb_dma(1, 0)
            vaug_dma(3)
            for qt in range(1, 4):
                eb_dma(1, qt)
            nc.sync.dma_start(out=kT_t[1][:], in_=kT_d[P:2 * P, :])
            nc.sync.dma_start(out=qT_t[1][:], in_=qT_d[P:2 * P, :])
            for qt in range(4):
                eb_dma(2, qt)

            touched = set([0, 1, 2])

            def on_head_start(h):
                hp2 = h + 3
                if hp2 < H and hp2 not in touched:
                    touched.add(hp2)
                    for qt in range(4):
                        eb_dma(hp2, qt)
                if h % 2 == 1 and h // 2 + 2 < NDIN:
                    c = h // 2 + 2
                    nc.sync.dma_start(out=kT_t[c][:], in_=kT_d[c * P:(c + 1) * P, :])
                    nc.sync.dma_start(out=qT_t[c][:], in_=qT_d[c * P:(c + 1) * P, :])
                if 3 <= h <= 6:
                    c = h - 3
                    nc.sync.dma_start(out=wT_t[c][:], in_=wT_d[c * P:(c + 1) * P, :])

            with (
                tc.tile_pool(name="scA", bufs=1, space="PSUM") as scpsA,
                tc.tile_pool(name="scB", bufs=1, space="PSUM") as scpsB,
                tc.tile_pool(name="avps", bufs=3, space="PSUM") as avps,
            ):
                # HAM warm-up: dummy matmuls un-throttle the PE clock
                # (4/8 -> 8/8) while the first DMAs land.
                wm_ps = scpsB.tile([P, 2 * T], F32, name="wm", tag="scB")
                for _ in range(10):
                    nc.tensor.matmul(wm_ps[:, 0:T], warm_t[:, 0:P], warm_t[:],
                                     start=True, stop=True)

                expv_w = [None] * NWIN
                av_t = {}
                pending = []   # heads awaiting bcast+normalize
                chain = {}     # h -> rcp tile

                def emit_qk(w):
                    g0, n = WINS[w]
                    width = 1536 if w % 2 == 0 else 1024
                    pool = scpsA if w % 2 == 0 else scpsB
                    sc_t = pool.tile([P, width], F32, name=f"sc{w}",
                                     tag=f"sc{'A' if w % 2 == 0 else 'B'}")
                    for j in range(n):
                        g = g0 + j
                        h, sc = g // 8, g % 8
                        if sc == 0:
                            on_head_start(h)
                        c2, half = divmod(h, 2)
                        hp = slice(half * HD, (half + 1) * HD)
                        nc.tensor.matmul(
                            sc_t[:, j * T:(j + 1) * T],
                            kT_t[c2][hp, sc * P:(sc + 1) * P],
                            qT_t[c2][hp, :],
                            start=True, stop=True,
                        )
                    eo = eopool.tile([P, 1536], F16, name=f"eo{w}", tag="eo")
                    nc.scalar.activation(
                        eo[:, 0:n * T], sc_t[:, 0:n * T], AF.Exp,
                        bias=eshift_t[:], scale=1.0,
                    )
                    # expv = exp(scores) * exp(bias): one DVE op per
                    # contiguous same-head run of chunks
                    ev = evpool.tile([P, 1536], F16, name=f"ev{w}", tag="ev")
                    expv_w[w] = ev
                    j = 0
                    while j < n:
                        h = (g0 + j) // 8
                        j2 = j
                        while j2 < n and (g0 + j2) // 8 == h:
                            j2 += 1
                        sl = slice(j * T, j2 * T)
                        bsl = slice((g0 + j) % NS * T, ((g0 + j) % NS + j2 - j) * T)
                        nc.vector.tensor_mul(ev[:, sl], eo[:, sl], eb_t[h][:, bsl])
                        j = j2

                def emit_bcast_flush(bc_pool=None):
                    while pending:
                        h = pending.pop(0)
                        rcp = chain.pop(h)
                        av = av_t[h]
                        rcp16 = spool.tile([1, T], F16, name=f"rh{h}", tag="rcp16")
                        nc.vector.tensor_copy(rcp16[:], rcp[:])
                        if bc_pool is None:
                            bc = avps.tile([P, T], F32, name=f"bc{h}", tag="avbc")
                        else:
                            bc = bc_pool.tile([P, 2 * T], F32, name=f"bc{h}", tag="scB")
                        nc.tensor.matmul(
                            bc[0:HD, :], ones_t[0:1, :], rcp16[:],
                            start=True, stop=True,
                        )
                        bc_sb = spool.tile([HD, T], F32, name=f"bcs{h}", tag="bc_sb")
                        if h >= 6:
                            nc.scalar.copy(bc_sb[:], bc[0:HD, :])
                        else:
                            nc.vector.tensor_copy(bc_sb[:], bc[0:HD, :])
                        c2, half = divmod(h, 2)
                        hp = slice(half * HD, (half + 1) * HD)
                        nc.vector.tensor_mul(aflat_t[c2][hp, :], av[0:HD, :], bc_sb[:])

                def emit_av(w):
                    g0, n = WINS[w]
                    for j in range(n):
                        g = g0 + j
                        h, sc = g // 8, g % 8
                        if sc == 0:
                            av_t[h] = avps.tile([P, T], F32, name=f"av{h}", tag="avbc")
                        nc.tensor.matmul(
                            av_t[h][0:HD + 1, :],
                            vaug_t[:, (sc * H + h) * (HD + 1):(sc * H + h + 1) * (HD + 1)],
                            expv_w[w][:, j * T:(j + 1) * T],
                            start=(sc == 0), stop=(sc == NS - 1),
                            skip_group_check=True,
                        )
                        if sc == NS - 1:
                            den_sb = spool.tile([1, T], F32, name=f"dn{h}", tag="den")
                            if h >= 6:
                                nc.scalar.copy(den_sb[:], av_t[h][HD:HD + 1, :])
                            else:
                                nc.vector.tensor_copy(den_sb[:], av_t[h][HD:HD + 1, :])
                            rcp = spool.tile([1, T], F32, name=f"rc{h}", tag="rcp")
                            nc.vector.reciprocal_approx_fast(rcp[:], den_sb[:])
                            chain[h] = rcp
                            pending.append(h)

                def keep_alive(n):
                    # tiny FD=128 dummies keep the HAM clock gate at 8/8
                    for _ in range(n):
                        nc.tensor.matmul(wm_ps[:, 0:P], warm_t[:, 0:P],
                                         warm_t[:, 0:P], start=True, stop=True)

                for w in range(NWIN):
                    if w % 2 == 1:
                        keep_alive(4)
                    emit_qk(w)
                    if w >= AV_LAG:
                        emit_av(w - AV_LAG)
                    emit_bcast_flush()

                # ---- out_proj drain, interleaved with the last head's
                # AV + normalize. Accumulators live in the freed score banks:
                # scA tile = dc0-2, scB tile = dc3-4, avps tiles = dc5-7.
                oA = scpsA.tile([P, 1536], F32, name="oA", tag="scA")
                oB = scpsB.tile([P, 2 * T], F32, name="oB", tag="scB")
                o_sl = {}
                for dc in range(3):
                    o_sl[dc] = oA[:, dc * T:(dc + 1) * T]
                for dc in (3, 4):
                    o_sl[dc] = oB[:, (dc - 3) * T:(dc - 2) * T]

                def emit_partials(dc):
                    for dinc in range(NDIN - 1):
                        nc.tensor.matmul(
                            o_sl[dc],
                            wT_t[dinc][:, dc * P:(dc + 1) * P],
                            aflat_t[dinc][:],
                            start=(dinc == 0), stop=False,
                            skip_group_check=True,
                        )

                emit_partials(0)                   # fills PE during exp(25)
                emit_av(NWIN - 1)                  # last AV chunk + den/rcp
                emit_partials(1)
                emit_partials(2)
                emit_partials(3)
                emit_partials(4)
                emit_bcast_flush()                 # h7: cast/bcast/evac/aflat
                for dc in (5, 6, 7):
                    o_sl[dc] = avps.tile([P, T], F32, name=f"o{dc}", tag="avbc")[:]
                    emit_partials(dc)
                for dc in range(ND):
                    nc.tensor.matmul(
                        o_sl[dc],
                        wT_t[NDIN - 1][:, dc * P:(dc + 1) * P],
                        aflat_t[NDIN - 1][:],
                        start=False, stop=True,
                        skip_group_check=True,
                    )
                    osb = opool_sb.tile([P, T], F16, name=f"osb{dc}", tag="osb")
                    if dc % 2 == 0:
                        nc.scalar.copy(osb[:], o_sl[dc])
                    else:
                        nc.vector.tensor_copy(osb[:], o_sl[dc])
                    nc.sync.dma_start(
                        out=outT_d[dc * P:(dc + 1) * P, :],
                        in_=osb[:],
                    )

    nc.finalize()
    return nc



_NC = None


def _get_nc():
    global _NC
    if _NC is None:
        _NC = build_bass()
    return _NC


def _core_index(b, tc_i, hg):
    return b * 4 + tc_i * 2 + hg


def _make_in_maps(query, key, value, attn_bias, key_padding_mask, out_w, out_b):
    query = np.asarray(query, dtype=np.float32)
    key = np.asarray(key, dtype=np.float32)
    value = np.asarray(value, dtype=np.float32)
    attn_bias = np.asarray(attn_bias, dtype=np.float32)
    mask = np.asarray(key_padding_mask).astype(bool)
    out_w = np.asarray(out_w, dtype=np.float32)

    wT_full = np.ascontiguousarray(out_w.T).astype(NP16)   # [din, dout]

    maps = [None] * 8
    for b in range(2):
        kT_full = np.ascontiguousarray(key[b].T).astype(NP16)  # [1024, 1024]
        for hg in range(2):
            hs = hg * H              # first global head of the group
            ds = hg * DIN            # first d_model row of the group
            vaug = np.ones((S, H * (HD + 1)), NP16)
            vaug.reshape(S, H, HD + 1)[:, :, :HD] = (
                value[b, :, ds:ds + DIN].reshape(S, H, HD))
            kT = np.ascontiguousarray(kT_full[ds:ds + DIN])
            wT = np.ascontiguousarray(wT_full[ds:ds + DIN])
            for tc_i in range(2):
                t0 = tc_i * T
                qT = np.ascontiguousarray(
                    (query[b, t0:t0 + T, ds:ds + DIN].T * SCALE)).astype(NP16)
                ebT = np.ascontiguousarray(
                    np.clip(attn_bias[b, hs:hs + H, t0:t0 + T, :], -10.0, 10.0)
                    .transpose(0, 2, 1))
                ebT[:, mask[b], :] = -np.inf
                np.exp(ebT, out=ebT)      # masked rows -> exactly 0
                maps[_core_index(b, tc_i, hg)] = {
                    "qT": qT, "kT": kT, "vaug": vaug,
                    "ebT": ebT.astype(NP16),
                    "wT": wT,
                }
    return maps


def run(inputs, trace=False, **run_kwargs):
    """Returns (output [2,1024,1024] f32, BassKernelResults)."""
    nc = _get_nc()
    inputs_out_b = inputs["out_b"]
    in_maps = _make_in_maps(**inputs)
    res = run_bass_kernel_spmd(
        nc, in_maps, core_ids=list(range(8)), trace=trace, **run_kwargs
    )
    out = np.empty((2, S, DM), np.float32)
    for b in range(2):
        for tc_i in range(2):
            part = (np.asarray(res.results[_core_index(b, tc_i, 0)]["outT"], dtype=np.float32)
                    + np.asarray(res.results[_core_index(b, tc_i, 1)]["outT"], dtype=np.float32))
            out[b, tc_i * T:(tc_i + 1) * T, :] = part.T + np.asarray(inputs_out_b, dtype=np.float32)[None, :]
    return out, res


def kernel(**inputs):
    out, _ = run(inputs, trace=False)
    return out
